# revision 46
# baseline (speedup 1.0000x reference)
"""Trainium2 Bass kernel for nn_Dynamics (GNN message passing), v7.

Data-parallel over batch n=1024 across 8 NeuronCores (NP=128 rows each).
Channel-major activations: (channels, batch*obj); every Linear is a PE matmul
with the (fan_in, fan_out) weight as lhsT.

Pair stage: one batch row nh (1024 pairs = 2 x 512-pair blocks) per iteration,
5 PE passes per block:
  p1 (3): Wi.T x_i + Wj.T x_j + wd.T dist -> (128, 512), relu evac (Act)
  p2 (1): [rel_w1 | att_w1] -> (64, 512) = [rel2; att2], relu evac (DVE, bf16)
  p3 (1): att_w2 replicated to M=33      -> (33, 512), exp evac (Act, bf16)
The rel L3 is factored out of the pair loop:
  rel_dyn = (rel_w2 + I).T q + rel_b2 * A,  q = sum_j rel2*att, A = sum_j att
prod = rel2*att is written in place over attW rows 0:32 (Pool, bf16); row 32
keeps the replicated att, so ONE segmented reduce per nh yields q rows 0:32
and A at row 32. Diag masking is a stride-33 memset on attW. The 4 loop
stages are software-pipelined one iteration apart (reverse emission order) so
no engine queue ever waits on a same-iteration cross-engine dependency.
"""

import numpy as np
from contextlib import ExitStack

import concourse.bass as bass
import concourse.mybir as mybir
import concourse.tile as tile
from concourse import bacc

F32 = mybir.dt.float32
F32R = mybir.dt.float32r
BF16 = mybir.dt.bfloat16
AF = mybir.ActivationFunctionType
ALU = mybir.AluOpType
AX = mybir.AxisListType

N = 1024
NOBJ = 32
CL = 32
NCORES = 8
NP = N // NCORES          # 128 batch rows per core
NO = NP * NOBJ            # 4096 objects per core
BLK = 512
NCHUNK = 32               # dist-repack chunks of 4 batch rows
CHUNK_N = NP // NCHUNK    # 4


def _build_program():
    nc = bacc.Bacc("TRN2", target_bir_lowering=False, debug=False)

    def din(name, shape):
        return nc.dram_tensor(name, list(shape), F32, kind="ExternalInput").ap()

    s3d = din("s3d", (NP, NOBJ, 16))
    wpack = din("wpack", (128, 940))
    out_d = nc.dram_tensor("out", [NP, NOBJ, CL], F32, kind="ExternalOutput").ap()

    with tile.TileContext(nc) as tc, ExitStack() as ctx:
        const = ctx.enter_context(tc.tile_pool(name="const", bufs=1))

        wp = const.tile([128, 940], F32, name="wp", tag="wp")
        nc.sync.dma_start(out=wp, in_=wpack)

        def conv(sl, rows, dt, nm):
            t = const.tile([rows, sl.stop - sl.start], dt, name=nm, tag=nm)
            nc.vector.tensor_copy(t, wp[0:rows, sl])
            return t

        t_ws2 = wp[:, 0:32]
        t_bs2 = wp[0:32, 32:33]
        t_wi = conv(slice(33, 161), 32, F32R, "t_wi")
        t_wj = conv(slice(161, 289), 32, F32R, "t_wj")
        t_wd = conv(slice(289, 417), 1, F32R, "t_wd")
        t_bl1 = wp[:, 417:418]
        t_w1c = conv(slice(418, 482), 128, F32R, "t_w1c")
        t_bl2 = wp[0:64, 482:483]
        t_w3a = conv(slice(483, 516), 64, F32R, "t_w3a")
        t_b3a = wp[0:33, 516:517]
        t_wdyn = conv(slice(517, 549), 33, F32R, "t_wdyn")
        t_wself0 = conv(slice(549, 581), 32, F32R, "t_wself0")
        t_bself0 = wp[0:32, 581:582]
        t_wself1i = conv(slice(582, 614), 32, F32R, "t_wself1i")
        t_bself1 = wp[0:32, 614:615]
        t_waff0 = conv(slice(615, 647), 32, F32R, "t_waff0")
        t_baff0 = wp[0:32, 647:648]
        t_waff1 = conv(slice(648, 680), 32, F32R, "t_waff1")
        t_baff1 = wp[0:32, 680:681]
        t_waff2 = conv(slice(681, 713), 32, F32R, "t_waff2")
        t_baff2 = wp[0:32, 713:714]
        t_wout0a = conv(slice(714, 746), 32, F32R, "t_wout0a")
        t_wout0b = conv(slice(746, 778), 32, F32R, "t_wout0b")
        t_bout0 = wp[0:32, 778:779]
        t_wout1i = conv(slice(779, 811), 32, F32R, "t_wout1i")
        t_bout1 = wp[0:32, 811:812]
        t_iden = wp[:, 812:940]

        # --- state load, padded to c=32 ---
        s_pad = const.tile([NP, NOBJ, 32], F32)
        nc.gpsimd.memset(s_pad, 0.0)
        nc.sync.dma_start(out=s_pad[:, :, 0:16], in_=s3d)

        pre_cm = tc.tile_pool(name="pre", bufs=1)
        pre = pre_cm.__enter__()
        pre_ps_cm = tc.tile_pool(name="pre_ps", bufs=2, space="PSUM")
        pre_ps = pre_ps_cm.__enter__()

        # --- s2 = concat(pos, enc[2:]) channel-major via PE transposes ---
        s2 = const.tile([32, NO], F32R)
        s_flat = s_pad.rearrange("p a b -> p (a b)")     # (128, 1024)
        s2_3d = s2.rearrange("p (n o) -> p n o", o=NOBJ)
        evac_rr = [0]

        def enc_evac(dst, src, bias):
            e = evac_rr[0] % 2
            evac_rr[0] += 1
            if e == 0:
                nc.scalar.activation(dst, src, AF.Identity, bias=bias)
            else:
                nc.vector.tensor_scalar(out=dst, in0=src, scalar1=bias,
                                        scalar2=None, op0=ALU.add)

        # --- pairwise squared distances, n on partitions ---
        pos = s_pad[:, :, 0:2]
        diff = pre.tile([NP, NOBJ, NOBJ, 2], F32)
        nc.vector.tensor_tensor(
            out=diff,
            in0=pos.unsqueeze(2).broadcast_to([NP, NOBJ, NOBJ, 2]),
            in1=pos.unsqueeze(1).broadcast_to([NP, NOBJ, NOBJ, 2]),
            op=ALU.subtract,
        )
        sq = pre.tile([NP, NOBJ, NOBJ, 2], F32)
        nc.scalar.activation(sq, diff, AF.Square)
        dist_t = const.tile([NP, NOBJ, NOBJ], F32)
        nc.vector.tensor_reduce(dist_t, sq, AX.X, ALU.add)

        for k in range(8):
            pst = pre_ps.tile([128, 128], F32, tag="ps_tr", name="pst")
            nc.tensor.transpose(pst, s_flat[:, 128 * k:128 * (k + 1)], t_iden)
            xt = pre.tile([128, 128], F32, tag="xt", bufs=3, name="xt")
            nc.vector.tensor_copy(xt, pst)
            for ol in range(4):
                o = 4 * k + ol
                ps = pre_ps.tile([32, 128], F32, tag="ps_enc", bufs=4, name="ps_enc")
                nc.tensor.matmul(ps, t_ws2[32 * ol:32 * ol + 16],
                                 xt[32 * ol:32 * ol + 16],
                                 start=True, stop=True,
                                 tile_position=(32 * ol, 0))
                enc_evac(s2_3d[:, :, o], ps, t_bs2)

        h1 = const.tile([32, NO], F32R)

        pre_ps_cm.__exit__(None, None, None)
        pre_cm.__exit__(None, None, None)

        # --- q accumulator: rows 0:32 = q, row 32 = A; cols (nh, i) ---
        q_big = const.tile([33, NO], F32R)

        work_cm = tc.tile_pool(name="work", bufs=1)
        work = work_cm.__enter__()
        distp_cm = tc.tile_pool(name="distp", bufs=2)
        distp = distp_cm.__enter__()
        ps1_cm = tc.tile_pool(name="ps1", bufs=2, space="PSUM")
        ps1 = ps1_cm.__enter__()
        ps23_cm = tc.tile_pool(name="ps23", bufs=2, space="PSUM")
        ps23 = ps23_cm.__enter__()
        tail_ps_cm = tc.tile_pool(name="tailps", bufs=2, space="PSUM")
        tail_ps = tail_ps_cm.__enter__()
        dist_tiles = {}
        pair1s, pair2s, attWs = {}, {}, {}

        def stage1(nh):
            ch, nl = divmod(nh, CHUNK_N)
            if nl == 0:
                df = distp.tile([1, CHUNK_N * 1024], F32R, tag="dist_f",
                                name="dist_f")
                nc.sync.dma_start(
                    out=df,
                    in_=dist_t.bitcast(F32R)[CHUNK_N * ch: CHUNK_N * (ch + 1)]
                    .rearrange("p a b -> p (a b)"))
                dist_tiles[ch] = df
            dist_f = dist_tiles[ch]
            xj = s2[:, nh * 32: nh * 32 + 32]
            xj = xj.unsqueeze(1).broadcast_to([32, 16, NOBJ])
            pair1 = work.tile([128, 2 * BLK], F32R, tag="pair1", bufs=3,
                              name="pair1")
            for ih in range(2):
                pp = ps1.tile([128, 16, NOBJ], F32, tag="p1", name="p1")
                xi = s2[:, nh * 32 + 16 * ih: nh * 32 + 16 * ih + 16]
                xi = xi.unsqueeze(2).broadcast_to([32, 16, NOBJ])
                drow = dist_f[0:1, nl * 1024 + ih * BLK:
                              nl * 1024 + (ih + 1) * BLK]
                nc.tensor.matmul(pp, t_wi, xi, start=True, stop=False)
                nc.tensor.matmul(pp, t_wj, xj, start=False, stop=False)
                nc.tensor.matmul(pp.rearrange("p a b -> p (a b)"), t_wd,
                                 drow, start=False, stop=True)
                nc.scalar.activation(pair1[:, ih * BLK:(ih + 1) * BLK],
                                     pp.rearrange("p a b -> p (a b)"),
                                     AF.Relu, bias=t_bl1)
            pair1s[nh] = pair1

        def stage2(nh):
            pair1 = pair1s.pop(nh)
            pair2 = work.tile([64, 2 * BLK], F32R, tag="pair2", bufs=4,
                              name="pair2")
            for ih in range(2):
                p2 = ps23.tile([64, BLK], F32, tag="p2", name="p2")
                nc.tensor.matmul(p2, t_w1c,
                                 pair1[:, ih * BLK:(ih + 1) * BLK],
                                 start=True, stop=True)
                nc.vector.tensor_scalar(
                    out=pair2[:, ih * BLK:(ih + 1) * BLK], in0=p2,
                    scalar1=t_bl2, scalar2=0.0, op0=ALU.add, op1=ALU.max)
            pair2s[nh] = pair2

        def stage3(nh):
            pair2 = pair2s[nh]
            attW = work.tile([33, 2 * BLK], BF16, tag="attW", bufs=4,
                             name="attW")
            for ih in range(2):
                p3 = ps23.tile([33, BLK], F32, tag="p3", name="p3")
                nc.tensor.matmul(p3, t_w3a[32:64],
                                 pair2[32:64, ih * BLK:(ih + 1) * BLK],
                                 start=True, stop=True)
                nc.scalar.activation(attW[:, ih * BLK:(ih + 1) * BLK], p3,
                                     AF.Exp, bias=t_b3a)
            attWs[nh] = attW

        def stage4(nh):
            pair2 = pair2s.pop(nh)
            attW = attWs.pop(nh)
            # mask self-pairs: diag cols form one stride-33 sequence across
            # both blocks (A: 33*il, B: 512+33*il+16 = 33*(16+il)); then prod
            # in place over attW rows 0:32; row 32 keeps the replicated att
            nc.gpsimd.memset(attW[0:33, 0:2 * BLK:33], 0.0)
            nc.gpsimd.tensor_tensor(out=attW[0:32, :],
                                    in0=pair2[0:32, :].bitcast(F32),
                                    in1=attW[0:32, :], op=ALU.mult)
            with nc.allow_low_precision(reason="f32r accum for dyn matmul"):
                nc.vector.tensor_reduce(
                    q_big[:, nh * 32:(nh + 1) * 32].rearrange(
                        "p (x a) -> p x a", x=2),
                    attW.rearrange("p (x a b) -> p x a b", x=2, b=NOBJ),
                    AX.X, ALU.add)

        tail_state = {}

        def emit_h1():
            for blk in range(NO // BLK):
                ps = ps23.tile([32, BLK], F32, tag="p2", name="ps_h1")
                nc.tensor.matmul(ps, t_wself0, s2[:, blk * BLK:(blk + 1) * BLK],
                                 start=True, stop=True)
                if blk % 2 == 0:
                    nc.scalar.activation(h1[:, blk * BLK:(blk + 1) * BLK], ps,
                                         AF.Relu, bias=t_bself0)
                else:
                    nc.vector.tensor_scalar(
                        out=h1[:, blk * BLK:(blk + 1) * BLK], in0=ps,
                        scalar1=t_bself0, scalar2=0.0, op0=ALU.add, op1=ALU.max)

        def tail_stage(wave, k):
            """Stage k (0..6) of the tail for slabs [4*wave, 4*wave+4)."""
            slabs = list(range(4 * wave, 4 * wave + 4))
            sl = {s: slice(512 * s, 512 * (s + 1)) for s in slabs}
            st = tail_state.setdefault(wave, {
                "dyn_sb": {}, "aff1": {}, "t2": {}, "aff3": {}, "o1": {},
                "res_sb": {}})
            tags = (["p2", "p3", "tpsA", "tpsB"] if wave == 0
                    else ["p2", "p3", "p1", "tpsA"])

            def ps(s, nm):
                tg = tags[s % 4]
                pool = (tail_ps if tg.startswith("tps")
                        else ps1 if tg == "p1" else ps23)
                return pool.tile([32, BLK], F32,
                                 bufs=1 if tg.startswith("tps") else None,
                                 tag=tg, name=nm)

            dyn_sb = st["dyn_sb"]; aff1 = st["aff1"]; t2 = st["t2"]
            aff3 = st["aff3"]; o1 = st["o1"]; res_sb = st["res_sb"]
            if k == 0:
                for s in slabs:
                    p = ps(s, "dynp")
                    nc.tensor.matmul(p, t_wdyn, q_big[:, sl[s]],
                                     start=True, stop=False)
                    nc.tensor.matmul(p, t_wself1i, h1[:, sl[s]],
                                     start=False, stop=True)
                    dyn_sb[s] = work.tile([32, BLK], F32R, tag="dyn_sb",
                                          bufs=4, name=f"dyn_sb{s}")
                    nc.vector.tensor_scalar(out=dyn_sb[s], in0=p,
                                            scalar1=t_bself1,
                                            scalar2=None, op0=ALU.add)
            elif k == 1:
                for s in slabs:
                    p = ps(s, "a1p")
                    nc.tensor.matmul(p, t_waff0, dyn_sb[s],
                                     start=True, stop=True)
                    aff1[s] = work.tile([32, BLK], F32R, tag="aff1", bufs=4,
                                        name=f"aff1{s}")
                    nc.scalar.activation(aff1[s], p, AF.Tanh, bias=t_baff0)
            elif k == 2:
                for s in slabs:
                    p = ps(s, "a2p")
                    nc.tensor.matmul(p, t_waff1, aff1[s],
                                     start=True, stop=True)
                    t2[s] = work.tile([32, BLK], F32R, tag="t2", bufs=4,
                                      name=f"t2_{s}")
                    nc.scalar.activation(t2[s], p, AF.Tanh, bias=t_baff1)
            elif k == 3:
                for s in slabs:
                    p = ps(s, "a3p")
                    nc.tensor.matmul(p, t_waff2, t2[s], start=True, stop=False)
                    nc.tensor.matmul(p, t_waff2, aff1[s],
                                     start=False, stop=True)
                    aff3[s] = work.tile([32, BLK], F32R, tag="aff3", bufs=4,
                                        name=f"aff3{s}")
                    nc.vector.tensor_scalar(out=aff3[s], in0=p,
                                            scalar1=t_baff2,
                                            scalar2=None, op0=ALU.add)
            elif k == 4:
                for s in slabs:
                    p = ps(s, "o1p")
                    nc.tensor.matmul(p, t_wout0a, aff3[s],
                                     start=True, stop=False)
                    nc.tensor.matmul(p, t_wout0b, s2[:, sl[s]],
                                     start=False, stop=True)
                    o1[s] = work.tile([32, BLK], F32R, tag="o1", bufs=4,
                                      name=f"o1_{s}")
                    nc.scalar.activation(o1[s], p, AF.Tanh, bias=t_bout0)
            elif k == 5:
                for s in slabs:
                    p = ps(s, "rp")
                    nc.tensor.matmul(p, t_wout1i, o1[s], start=True, stop=True)
                    res_sb[s] = work.tile([32, BLK], F32, tag="res_sb",
                                          bufs=4, name=f"res{s}")
                    nc.vector.tensor_scalar(out=res_sb[s], in0=p,
                                            scalar1=t_bout1,
                                            scalar2=None, op0=ALU.add)
            else:
                for s in slabs:
                    for t in range(4):
                        tg = tags[s % 4]
                        pool = (tail_ps if tg.startswith("tps")
                                else ps1 if tg == "p1" else ps23)
                        pstt = pool.tile([128, BLK], F32,
                                         bufs=1 if tg.startswith("tps")
                                         else None, tag=tg, name="pstt")
                        pst2 = pstt[:, 0:32]
                        nc.tensor.transpose(
                            pst2, res_sb[s][:, 128 * t:128 * (t + 1)],
                            t_iden[0:32, 0:32])
                        o_sb = work.tile([128, 32], F32, tag="o_sb", bufs=4,
                                         name="o_sb")
                        nc.vector.tensor_copy(o_sb, pst2)
                        r0 = 16 * s + 4 * t
                        nc.sync.dma_start(
                            out=out_d[r0:r0 + 4].rearrange("a o c -> (a o) c"),
                            in_=o_sb)

        for t in range(NP + 3):
            if 0 <= t - 2 < NP:
                stage3(t - 2)
            if 0 <= t - 1 < NP:
                stage2(t - 1)
            if 0 <= t - 3 < NP:
                stage4(t - 3)
            if t < NP:
                stage1(t)
            if t == 30:
                emit_h1()
            if t == 67:
                for k in range(7):
                    tail_stage(0, k)

        for k in range(7):
            tail_stage(1, k)

        tail_ps_cm.__exit__(None, None, None)
        ps23_cm.__exit__(None, None, None)
        ps1_cm.__exit__(None, None, None)
        distp_cm.__exit__(None, None, None)
        work_cm.__exit__(None, None, None)

    nc.compile()
    return nc


_PROG = None


def _get_program():
    global _PROG
    if _PROG is None:
        _PROG = _build_program()
    return _PROG


def _prep_weights(inp):
    g = lambda k: np.asarray(inp[k], dtype=np.float32)
    w_s2_base = g("state_enc_w").copy()
    b_s2 = g("state_enc_b").copy()
    # s2 keeps raw channels 0-1
    w_s2_base[:, 0:2] = 0.0
    w_s2_base[0, 0] = 1.0
    w_s2_base[1, 1] = 1.0
    b_s2[0:2] = 0.0
    w_s2 = np.zeros((128, 32), np.float32)
    for q in range(4):
        w_s2[32 * q:32 * q + 16] = w_s2_base

    rel_w0, att_w0 = g("rel_w0"), g("att_w0")
    wi_cat = np.concatenate([rel_w0[:32], att_w0[:32]], axis=1)
    wj_cat = np.concatenate([rel_w0[32:64], att_w0[32:64]], axis=1)
    wd_cat = np.concatenate([rel_w0[64:65], att_w0[64:65]], axis=1)
    b_l1 = np.concatenate([g("rel_b0"), g("att_b0")])

    w1_cat = np.zeros((128, 64), np.float32)
    w1_cat[0:64, 0:32] = g("rel_w1")
    w1_cat[64:128, 32:64] = g("att_w1")
    b_l2 = np.concatenate([g("rel_b1"), g("att_b1")])

    w3att = np.zeros((64, 33), np.float32)
    att_w2 = g("att_w2").reshape(-1)          # (32,)
    w3att[32:64, :] = att_w2[:, None]

    wdyn = np.zeros((33, 32), np.float32)
    wdyn[0:32] = g("rel_w2") + np.eye(32, dtype=np.float32)
    wdyn[32] = g("rel_b2")

    wpack = np.zeros((128, 940), np.float32)

    def put(col0, arr):
        arr = np.asarray(arr, np.float32)
        if arr.ndim == 1:
            arr = arr.reshape(-1, 1)
        wpack[0:arr.shape[0], col0:col0 + arr.shape[1]] = arr
        return col0 + arr.shape[1]

    c = put(0, w_s2)
    c = put(c, b_s2)
    c = put(c, wi_cat)
    c = put(c, wj_cat)
    c = put(c, wd_cat)
    c = put(c, b_l1)
    c = put(c, w1_cat)
    c = put(c, b_l2)
    c = put(c, w3att)
    c = put(c, np.full(33, float(g("att_b2").reshape(-1)[0]), np.float32))
    c = put(c, wdyn)
    c = put(c, g("self_w0")); c = put(c, g("self_b0"))
    c = put(c, g("self_w1") + np.eye(32, dtype=np.float32))
    c = put(c, g("self_b1"))
    c = put(c, g("aff_w0")); c = put(c, g("aff_b0"))
    c = put(c, g("aff_w1")); c = put(c, g("aff_b1"))
    c = put(c, g("aff_w2")); c = put(c, g("aff_b2"))
    c = put(c, g("out_w0")[0:32]); c = put(c, g("out_w0")[32:64])
    c = put(c, g("out_b0"))
    c = put(c, g("out_w1") + np.eye(32, dtype=np.float32))
    c = put(c, g("out_b1"))
    c = put(c, np.eye(128, dtype=np.float32))
    assert c == 940, c
    return {"wpack": wpack}


def kernel(**inputs) -> np.ndarray:
    from concourse.bass_utils import run_bass_kernel_spmd

    nc = _get_program()
    weights = _prep_weights(inputs)
    s = np.asarray(inputs["s"], dtype=np.float32)
    in_maps = []
    for core in range(NCORES):
        m = dict(weights)
        m["s3d"] = np.ascontiguousarray(s[core * NP:(core + 1) * NP])
        in_maps.append(m)
    res = run_bass_kernel_spmd(nc, in_maps, list(range(NCORES)))
    out = np.concatenate([res.results[i]["out"] for i in range(NCORES)], axis=0)
    return out.astype(np.float32)
